# revision 1
# baseline (speedup 1.0000x reference)
"""Trainium2 Bass kernel for nn_Detector (batched FPS detector head).

Pipeline per core (256 submaps = 2 tiles of 128 submaps x 1024 points):
  1. Load pos interleaved, split channels to contiguous px/py/pz.
  2. v = px^2 + py^2 per point (exact f32, matches reference ranking).
  3. Per-submap exact 512-smallest threshold via 31-step bisection on the
     int32 bit pattern of v (positive floats are monotone as ints).
  4. Stream-compact the 512 selected points per submap: cumsum ranks +
     per-partition local_scatter of the f32 coords as uint16 halves.
     Compacted interleaved coords also go to a DRAM scratch so the FPS
     loop can gather per-partition center rows via indirect DMA.
  5. 32-point farthest-point sampling, vectorized across 128 submaps
     (points on the free dim), bit-exact d^2 arithmetic.
  6. Gather only the 32 selected x-rows per submap from DRAM and run the
     tiny MLP on them (block-diagonal weights, 4 lanes on partitions).
     softplus(z) = -ln(sigmoid(-z)) (no softplus ACT table in this build).

Outputs per core: weights [256, 32] f32, indices [256, 32] int32.
"""

import sys

for _p in ("/opt/trn_rl_repo",):
    if _p not in sys.path:
        sys.path.insert(0, _p)

import numpy as np

import concourse.bass as bass
import concourse.bacc as bacc
import concourse.mybir as mybir
from concourse.bass import IndirectOffsetOnAxis
from concourse.mybir import ActivationFunctionType as actf
from concourse.mybir import AluOpType as alu
from concourse.tile import TileContext

f32 = mybir.dt.float32
i32 = mybir.dt.int32
i16 = mybir.dt.int16
u16 = mybir.dt.uint16
u32 = mybir.dt.uint32

P = 128          # partitions = submaps per tile
NPTS = 1024      # points per submap
KPOS = 512       # closest points kept
K = 32           # FPS samples per submap
TILES = 2        # tiles per core
S_CORE = P * TILES  # submaps per core
N_CORES = 8
BISECT = 31      # int-bitspace bisection steps: 2^31 -> 1

INF_BITS = 0x7F800000


def build_nc():
    nc = bacc.Bacc()

    pos_in = nc.declare_dram_parameter("pos", [S_CORE * NPTS, 3], f32, isOutput=False)
    x_in = nc.declare_dram_parameter("x", [S_CORE * NPTS, 32], f32, isOutput=False)
    w1d_in = nc.declare_dram_parameter("W1d", [128, 64], f32, isOutput=False)
    w2d_in = nc.declare_dram_parameter("W2d", [64, 32], f32, isOutput=False)
    w3d_in = nc.declare_dram_parameter("W3d", [32, 4], f32, isOutput=False)
    b1d_in = nc.declare_dram_parameter("b1d", [64, 1], f32, isOutput=False)
    b2d_in = nc.declare_dram_parameter("b2d", [32, 1], f32, isOutput=False)
    b3d_in = nc.declare_dram_parameter("b3d", [4, 1], f32, isOutput=False)
    eye_in = nc.declare_dram_parameter("eye128", [128, 128], f32, isOutput=False)

    w_out = nc.declare_dram_parameter("weights_out", [S_CORE, K], f32, isOutput=True)
    i_out = nc.declare_dram_parameter("indices_out", [S_CORE, K], i32, isOutput=True)

    # DRAM scratch: compacted rows [x, y, z, orig_idx] per selected point
    pc_dram = [
        nc.dram_tensor(f"pc_dram{t}", [P * KPOS, 4], f32) for t in range(TILES)
    ]

    pos_t = pos_in[:].rearrange("(t p f) c -> t p (f c)", t=TILES, p=P)

    with TileContext(nc) as tc, tc.tile_pool(name="main", bufs=1) as pool:
        # ---- shared constants ----
        zeros8 = pool.tile([P, 8], f32, tag="zeros8")
        nc.vector.memset(zeros8[:], 0.0)
        eye = pool.tile([P, 128], f32, tag="eye")
        nc.sync.dma_start(out=eye[:], in_=eye_in[:])
        # row base offsets: p*512 (compacted rows) and global x-row base
        rb512 = pool.tile([P, 1], i32, tag="rb512")
        nc.gpsimd.iota(rb512[:], [[1, 1]], base=0, channel_multiplier=KPOS)
        rb512f = pool.tile([P, 1], f32, tag="rb512f")
        nc.vector.tensor_copy(rb512f[:], rb512[:])

        w1d = pool.tile([P, 64], f32, tag="w1d")
        nc.sync.dma_start(out=w1d[:], in_=w1d_in[:])
        w2d = pool.tile([64, 32], f32, tag="w2d")
        nc.sync.dma_start(out=w2d[:], in_=w2d_in[:])
        w3d = pool.tile([32, 4], f32, tag="w3d")
        nc.sync.dma_start(out=w3d[:], in_=w3d_in[:])
        b1d = pool.tile([64, 1], f32, tag="b1d")
        nc.sync.dma_start(out=b1d[:], in_=b1d_in[:])
        b2d = pool.tile([32, 1], f32, tag="b2d")
        nc.sync.dma_start(out=b2d[:], in_=b2d_in[:])
        b3d = pool.tile([4, 1], f32, tag="b3d")
        nc.sync.dma_start(out=b3d[:], in_=b3d_in[:])
        b3n = pool.tile([4, 1], f32, tag="b3n")
        nc.vector.tensor_scalar_mul(b3n[:], b3d[:], -1.0)
        iota16 = pool.tile([P, NPTS], i16, tag="iota16")
        nc.gpsimd.iota(iota16[:], [[1, NPTS]], channel_multiplier=0)
        zsigs = []

        # ---- load pos, split to contiguous channels, v = px^2+py^2 ----
        v, pch = [], []
        for t in range(TILES):
            pil = pool.tile([P, NPTS * 3], f32, tag=f"pos_il{t}", name=f"pos_il{t}")
            nc.sync.dma_start(out=pil[:], in_=pos_t[t])
            p3 = pil[:].rearrange("p (n c) -> p n c", c=3)
            chans = []
            for c, eng in ((0, nc.vector), (1, nc.gpsimd), (2, nc.scalar)):
                ch = pool.tile([P, NPTS], f32, tag=f"ch{c}_{t}", name=f"ch{c}_{t}")
                if eng is nc.scalar:
                    eng.copy(ch[:], p3[:, :, c])
                else:
                    eng.tensor_copy(ch[:], p3[:, :, c])
                chans.append(ch)
            pch.append(chans)
            sq = pool.tile([P, NPTS], f32, tag=f"sq{t}", name=f"sq{t}")
            nc.vector.tensor_tensor(sq[:], chans[0][:], chans[0][:], alu.mult)
            sqy = pool.tile([P, NPTS], f32, tag=f"sqy{t}", name=f"sqy{t}")
            nc.gpsimd.tensor_tensor(sqy[:], chans[1][:], chans[1][:], alu.mult)
            vt = pool.tile([P, NPTS], f32, tag=f"v{t}", name=f"v{t}")
            nc.vector.tensor_tensor(vt[:], sq[:], sqy[:], alu.add)
            v.append(vt)

        # ---- bisection for the 512th-smallest v ----
        # The DVE runs "int" ALU ops through its f32 datapath, so int bit
        # arithmetic at ~2^30 magnitude rounds. Two exact stages instead:
        #   A) 24 steps on the bit pattern restricted to multiples of 128
        #      (always exactly representable as f32 integers),
        #   B) ~10 steps of float-value bisection inside the 128-ulp window.
        # Per-tile bisection state so the two chains are independent: the
        # scheduler can overlap tile 0's compaction/FPS with tile 1's probes.
        bst = []
        for t in range(TILES):
            b = {}
            for nm, dt_ in (("lo", f32), ("hi", f32), ("mid", f32),
                            ("midi", i32), ("step", f32), ("cnt", f32),
                            ("cond", i32)):
                b[nm] = pool.tile([P, 1], dt_, tag=f"{nm}{t}", name=f"{nm}{t}")
            nc.vector.memset(b["lo"][:], 0.0)
            nc.vector.memset(b["hi"][:], float(2 ** 31))
            b["junk"] = pool.tile([P, NPTS], f32, tag=f"junk{t}", name=f"junk{t}")
            bst.append(b)

        def probe(t, thr_f32_view):
            # cnt <-> #{v_t <= thr}; float compare == bit compare for positive
            # floats. Tile 0 probes on ACT (Sign of the exact FMA subtraction,
            # sum-accumulated: S' = sum(sign(thr - v)); #le <= 512 iff
            # S' <= 0, normalized to the shared compare). Tile 1 on DVE is_le.
            b = bst[t]
            if t == 0:
                nc.scalar.activation(
                    b["junk"][:], v[0][:], actf.Sign,
                    bias=thr_f32_view, scale=-1.0, accum_out=b["cnt"][:],
                )
                nc.vector.tensor_scalar(
                    b["cnt"][:], b["cnt"][:], 0.5, 512.0, alu.mult, alu.add
                )
            else:
                nc.vector.tensor_scalar(
                    b["junk"][:], v[1][:], thr_f32_view, None,
                    alu.is_le, alu.add, accum_out=b["cnt"][:],
                )

        def update(t):
            # cnt <= 512 -> lo = mid else hi = mid
            b = bst[t]
            nc.vector.tensor_scalar(b["cond"][:], b["cnt"][:], 512.0, None, alu.is_le)
            nc.vector.copy_predicated(b["lo"][:], b["cond"][:], b["mid"][:])
            nc.vector.tensor_scalar(b["cond"][:], b["cnt"][:], 512.0, None, alu.is_gt)
            nc.vector.copy_predicated(b["hi"][:], b["cond"][:], b["mid"][:])

        def midpoint(t):
            b = bst[t]
            nc.vector.tensor_tensor(b["step"][:], b["hi"][:], b["lo"][:], alu.subtract)
            nc.vector.tensor_scalar_mul(b["step"][:], b["step"][:], 0.5)
            nc.vector.tensor_tensor(b["mid"][:], b["lo"][:], b["step"][:], alu.add)

        for t in range(TILES):
            b = bst[t]
            for it in range(24):  # stage A: bit grid of 128, spans 2^31 -> 128
                midpoint(t)
                nc.vector.tensor_copy(b["midi"][:], b["mid"][:])  # exact f32->i32
                probe(t, b["midi"][:].bitcast(f32))
                update(t)
            # stage B init: reinterpret the bit window as float values
            nc.vector.tensor_copy(b["midi"][:], b["lo"][:])
            nc.vector.tensor_copy(b["lo"][:].bitcast(i32), b["midi"][:])
            nc.vector.tensor_copy(b["midi"][:], b["hi"][:])
            nc.vector.tensor_copy(b["hi"][:].bitcast(i32), b["midi"][:])
            for it in range(8):  # stage B: value bisection, 128 ulps -> 1
                midpoint(t)
                probe(t, b["mid"][:])
                update(t)

        tau = [bst[t]["lo"] for t in range(TILES)]  # exact 512-smallest thr

        # ---- compact: mask -> ranks -> scatter coords/indices ----
        pcc, selc = [], []
        for t in range(TILES):
            tau_f = tau[t][:]
            mask = pool.tile([P, NPTS], f32, tag=f"mask{t}", name=f"mask{t}")
            nc.vector.tensor_scalar(mask[:], v[t][:], tau_f, None, alu.is_le)
            rank = pool.tile([P, NPTS], f32, tag=f"rank{t}", name=f"rank{t}")
            nc.vector.tensor_tensor_scan(
                rank[:], mask[:], mask[:], 0.0, alu.add, alu.bypass
            )
            # slot = rank * mask - 1  (-1 for unselected -> ignored by scatter)
            slot = pool.tile([P, NPTS], f32, tag=f"slot{t}", name=f"slot{t}")
            nc.vector.tensor_tensor(slot[:], rank[:], mask[:], alu.mult)
            nc.vector.tensor_scalar(slot[:], slot[:], -1.0, None, alu.add)

            # u16-half slot indices: even half -> 2*slot, odd half -> 2*slot+1
            idx2f = pool.tile([P, 2 * NPTS], f32, tag=f"idx2f{t}", name=f"idx2f{t}")
            i2v = idx2f[:].rearrange("p (n two) -> p n two", two=2)
            nc.vector.tensor_scalar_mul(i2v[:, :, 0], slot[:], 2.0)
            nc.vector.tensor_scalar(i2v[:, :, 1], slot[:], 2.0, 1.0, alu.mult, alu.add)
            idx2 = pool.tile([P, 2 * NPTS], i16, tag=f"idx2_{t}", name=f"idx2_{t}")
            nc.vector.tensor_copy(idx2[:], idx2f[:])

            # compacted coord channels (as u16 halves of f32)
            chc = []
            for c in range(3):
                cc = pool.tile([P, 2 * KPOS], u16, tag=f"cc{c}_{t}", name=f"cc{c}_{t}")
                nc.gpsimd.local_scatter(
                    cc[:], pch[t][c][:].bitcast(u16), idx2[:],
                    channels=P, num_elems=2 * KPOS, num_idxs=2 * NPTS,
                )
                chc.append(cc)
            pcc.append([cc[:].bitcast(f32) for cc in chc])

            # compacted original indices (i16 scatter of iota, then widen)
            slot16 = pool.tile([P, NPTS], i16, tag=f"slot16{t}", name=f"slot16{t}")
            nc.vector.tensor_copy(slot16[:], slot[:])
            sel16 = pool.tile([P, KPOS], i16, tag=f"sel16{t}", name=f"sel16{t}")
            nc.gpsimd.local_scatter(
                sel16[:], iota16[:], slot16[:],
                channels=P, num_elems=KPOS, num_idxs=NPTS,
            )
            self_f = pool.tile([P, KPOS], f32, tag=f"selff{t}", name=f"selff{t}")
            nc.vector.tensor_copy(self_f[:], sel16[:])
            selc.append(self_f)

            # interleave compacted rows [x,y,z,orig] -> DRAM for row gathers
            pci = pool.tile([P, KPOS * 4], f32, tag=f"pci{t}", name=f"pci{t}")
            pciv = pci[:].rearrange("p (n c) -> p n c", c=4)
            nc.vector.tensor_copy(pciv[:, :, 0], pcc[t][0])
            nc.vector.tensor_copy(pciv[:, :, 1], pcc[t][1])
            nc.vector.tensor_copy(pciv[:, :, 2], pcc[t][2])
            nc.vector.tensor_copy(pciv[:, :, 3], self_f[:])
            nc.sync.dma_start(
                out=pc_dram[t][:].rearrange("(p n) c -> p (n c)", p=P),
                in_=pci[:],
            )

        # ---- FPS: both tiles interleaved per iteration ----
        st = []
        for t in range(TILES):
            s = {}
            s["pxc"], s["pyc"], s["pzc"] = pcc[t]
            s["m8"] = pool.tile([P, 8], f32, tag=f"m8_{t}", name=f"m8_{t}")
            s["pidx"] = pool.tile([P, 8], u32, tag=f"pidx_{t}", name=f"pidx_{t}")
            s["pf"] = pool.tile([P, 1], f32, tag=f"pf_{t}", name=f"pf_{t}")
            s["orig32"] = pool.tile([P, K], i32, tag=f"orig32_{t}", name=f"orig32_{t}")
            s["c4"] = pool.tile([P, 4], f32, tag=f"c4_{t}", name=f"c4_{t}")
            s["cneg"] = pool.tile([P, 3], f32, tag=f"cneg_{t}", name=f"cneg_{t}")
            s["goff"] = pool.tile([P, 1], f32, tag=f"goff_{t}", name=f"goff_{t}")
            s["goffi"] = pool.tile([P, 1], i32, tag=f"goffi_{t}", name=f"goffi_{t}")
            s["gxf"] = pool.tile([P, 1], f32, tag=f"gxf_{t}", name=f"gxf_{t}")
            s["gxi"] = pool.tile([P, 1], i32, tag=f"gxi_{t}", name=f"gxi_{t}")
            s["xg"] = pool.tile([P, K * 32], f32, tag=f"pos_il{t}", name=f"xg_{t}")
            s["dx"] = pool.tile([P, KPOS], f32, tag=f"mask{t}", name=f"dx_{t}")
            s["dy"] = pool.tile([P, KPOS], f32, tag=f"rank{t}", name=f"dy_{t}")
            s["dz"] = pool.tile([P, KPOS], f32, tag=f"slot{t}", name=f"dz_{t}")
            s["sqx"] = pool.tile([P, KPOS], f32, tag=f"sq{t}", name=f"sqx_{t}")
            s["sqy2"] = pool.tile([P, KPOS], f32, tag=f"sqy{t}", name=f"sqy2_{t}")
            s["sqz"] = pool.tile([P, KPOS], f32, tag=f"slot16{t}", name=f"sqz_{t}")
            s["s1"] = pool.tile([P, KPOS], f32, tag=f"ch0_{t}", name=f"s1_{t}")
            s["d2"] = pool.tile([P, KPOS], f32, tag=f"ch1_{t}", name=f"d2_{t}")
            s["md"] = [
                pool.tile([P, KPOS], f32, tag=f"ch2_{t}", name=f"mdA_{t}"),
                pool.tile([P, KPOS], f32, tag=f"idx2_{t}", name=f"mdB_{t}"),
            ]
            st.append(s)

        def dist2(s, out_ap):
            # exact f32: ((px-cx)^2 + (py-cy)^2) + (pz-cz)^2
            # subs on ACT (exact FMA), squares split DVE/GPSIMD, sums GPSIMD
            cneg = s["cneg"]
            nc.scalar.activation(
                s["dx"][:], s["pxc"], actf.Identity, bias=cneg[:, 0:1], scale=1.0
            )
            nc.scalar.activation(
                s["dy"][:], s["pyc"], actf.Identity, bias=cneg[:, 1:2], scale=1.0
            )
            nc.vector.tensor_scalar(s["dz"][:], s["pzc"], cneg[:, 2:3], None, alu.add)
            nc.gpsimd.tensor_tensor(s["sqx"][:], s["dx"][:], s["dx"][:], alu.mult)
            nc.vector.tensor_tensor(s["sqy2"][:], s["dy"][:], s["dy"][:], alu.mult)
            nc.vector.tensor_tensor(s["sqz"][:], s["dz"][:], s["dz"][:], alu.mult)
            nc.gpsimd.tensor_tensor(s["s1"][:], s["sqx"][:], s["sqy2"][:], alu.add)
            nc.gpsimd.tensor_tensor(out_ap, s["s1"][:], s["sqz"][:], alu.add)

        for t in range(TILES):
            s = st[t]
            # start = argmin v (the closest selected point); max8 + max_index
            # on -v; first occurrence wins as in jnp.argmax.
            vneg = pool.tile([P, NPTS], f32, tag=f"junk{t}", name=f"vneg_{t}")
            nc.vector.tensor_scalar_mul(vneg[:], v[t][:], -1.0)
            nc.vector.max(s["m8"][:], vneg[:])
            nc.vector.max_index(s["pidx"][:], s["m8"][:], vneg[:])
            nc.vector.tensor_copy(s["orig32"][:, 0:1], s["pidx"][:, 0:1])

            # gather start coords + x row via original row (t*128+p)*1024+idx
            xrow = pool.tile([P, 1], i32, tag=f"xrow_{t}", name=f"xrow_{t}")
            nc.gpsimd.iota(
                xrow[:], [[1, 1]], base=t * P * NPTS, channel_multiplier=NPTS
            )
            xrowf = pool.tile([P, 1], f32, tag=f"xrowf_{t}", name=f"xrowf_{t}")
            nc.vector.tensor_copy(xrowf[:], xrow[:])
            s["xrowf"] = xrowf
            nc.vector.tensor_copy(s["pf"][:], s["pidx"][:, 0:1])
            nc.vector.tensor_scalar(s["goff"][:], s["pf"][:], xrowf[:], None, alu.add)
            nc.vector.tensor_copy(s["goffi"][:], s["goff"][:])
            nc.gpsimd.indirect_dma_start(
                out=s["c4"][:, 0:3],
                out_offset=None,
                in_=pos_in[:],
                in_offset=IndirectOffsetOnAxis(ap=s["goffi"][:], axis=0),
            )
            # min_d init: distance to start point. ACT bias needs -c.
            nc.vector.tensor_scalar_mul(s["cneg"][:], s["c4"][:, 0:3], -1.0)
            dist2(s, s["md"][0][:])

        for k in range(1, K):
            for t in range(TILES):
                s = st[t]
                src = s["md"][(k - 1) % 2]
                dst = s["md"][k % 2]
                # argmax of current min_d; first occurrence as in jnp.argmax
                nc.vector.max(s["m8"][:], src[:])
                nc.vector.max_index(s["pidx"][:], s["m8"][:], src[:])
                # gather [x, y, z, orig] of the pick from compacted DRAM rows
                nc.vector.tensor_scalar(
                    s["goff"][:], s["pidx"][:, 0:1], rb512f[:], None, alu.add
                )
                nc.vector.tensor_copy(s["goffi"][:], s["goff"][:])
                nc.gpsimd.indirect_dma_start(
                    out=s["c4"][:],
                    out_offset=None,
                    in_=pc_dram[t][:],
                    in_offset=IndirectOffsetOnAxis(ap=s["goffi"][:], axis=0),
                )
                nc.vector.tensor_copy(s["orig32"][:, k : k + 1], s["c4"][:, 3:4])
                nc.vector.tensor_scalar_mul(s["cneg"][:], s["c4"][:, 0:3], -1.0)
                dist2(s, s["d2"][:])
                # min_d' = min(min_d, d2)
                nc.vector.tensor_tensor(dst[:], src[:], s["d2"][:], alu.min)

        # ---- batched x-row gathers (off the FPS critical chain) ----
        for t in range(TILES):
            s = st[t]
            growf = pool.tile([P, K], f32, tag=f"growf_{t}", name=f"growf_{t}")
            nc.vector.tensor_copy(growf[:], s["orig32"][:])
            nc.vector.tensor_scalar(
                growf[:], growf[:], s["xrowf"][:], None, alu.add
            )
            grow = pool.tile([P, K], i32, tag=f"grow_{t}", name=f"grow_{t}")
            nc.vector.tensor_copy(grow[:], growf[:])
            s["grow"] = grow
        for k in range(K):
            for t in range(TILES):
                s = st[t]
                nc.gpsimd.indirect_dma_start(
                    out=s["xg"][:, k * 32 : (k + 1) * 32],
                    out_offset=None,
                    in_=x_in[:],
                    in_offset=IndirectOffsetOnAxis(ap=s["grow"][:, k : k + 1], axis=0),
                )

        # ---- MLP per tile ----
        origs = []
        for t in range(TILES):
            s = st[t]
            orig32, xg = s["orig32"], s["xg"]
            xg3 = xg[:].rearrange("p (k f) -> p k f", f=32)
            nc.sync.dma_start(out=i_out[t * P : (t + 1) * P, :], in_=orig32[:])
            origs.append(orig32)

            # transpose 32 chunks of [128, 32] -> xT4 [128 (4 lanes x 32 feat), 1024]
            with tc.tile_pool(name=f"psum{t}", bufs=1, space="PSUM") as psp:
                ps_xt = psp.tile([P, 1024], f32, tag=f"psA{t}")
                for j in range(K):
                    lane, grp = j % 4, j // 4
                    # transpose as a plain matmul (chunk.T @ I): walrus only
                    # allows transpose-mode outputs at PSUM partition 0.
                    nc.tensor.matmul(
                        ps_xt[lane * 32 : (lane + 1) * 32, grp * 128 : (grp + 1) * 128],
                        xg3[:, j, :],
                        eye[:],
                        tile_position=(0, lane * 32),
                    )
                xt4 = pool.tile([P, 1024], f32, tag=f"idx2f{t}", name=f"xt4_{t}")
                nc.scalar.copy(xt4[:], ps_xt[:])

                ps_h = psp.tile([64, 1024], f32, tag=f"psB{t}")
                for c in range(2):
                    nc.tensor.matmul(
                        ps_h[:, c * 512 : (c + 1) * 512],
                        w1d[:], xt4[:, c * 512 : (c + 1) * 512],
                    )
                h1 = pool.tile([64, 1024], f32, tag=f"v{t}", name=f"h1_{t}")
                nc.scalar.activation(h1[:], ps_h[:], actf.Relu, bias=b1d[:], scale=1.0)

                ps_h2 = psp.tile([32, 1024], f32, tag=f"psA{t}")
                for c in range(2):
                    nc.tensor.matmul(
                        ps_h2[:, c * 512 : (c + 1) * 512],
                        w2d[:], h1[:, c * 512 : (c + 1) * 512],
                    )
                h2 = pool.tile([32, 1024], f32, tag=f"mask{t}", name=f"h2_{t}")
                nc.scalar.activation(h2[:], ps_h2[:], actf.Relu, bias=b2d[:], scale=1.0)

                ps_z = psp.tile([4, 1024], f32, tag=f"psB{t}")
                for c in range(2):
                    nc.tensor.matmul(
                        ps_z[:, c * 512 : (c + 1) * 512],
                        w3d[:], h2[:, c * 512 : (c + 1) * 512],
                    )
                # softplus(z) = -ln(sigmoid(-z)); no softplus table here.
                zsig = pool.tile([4, 1024], f32, tag=f"zsig_{t}", name=f"zsig{t}")
                nc.scalar.activation(
                    zsig[:], ps_z[:], actf.Sigmoid, bias=b3n[:], scale=-1.0
                )
                zsigs.append(zsig)

        # ---- tail: -ln(zsig), reorder [4,1024] -> [128,32], write out ----
        for t in range(TILES):
            s4 = pool.tile([4, 1024], f32, tag=f"rank{t}", name=f"s4_{t}")
            nc.scalar.activation(s4[:], zsigs[t][:], actf.Ln)
            with tc.tile_pool(name=f"psumw{t}", bufs=1, space="PSUM") as pspw:
                ps_w = pspw.tile([P, K], f32, tag=f"psW{t}")
                for c in range(8):
                    nc.tensor.transpose(
                        ps_w[:, c * 4 : (c + 1) * 4],
                        s4[:, c * 128 : (c + 1) * 128],
                        eye[0:4, 0:4],
                    )
                wout = pool.tile([P, K], f32, tag=f"wout_{t}", name=f"wout{t}")
                # wout = -ln(sigmoid(-z)) = softplus(z)
                nc.vector.tensor_scalar_mul(wout[:], ps_w[:], -1.0)
            nc.sync.dma_start(out=w_out[t * P : (t + 1) * P, :], in_=wout[:])

    nc.compile()
    return nc


def _host_prep(W1, b1, W2, b2, W3, b3):
    """Block-diagonal 4-lane weight stacks + replicated biases."""
    W1 = np.asarray(W1, np.float32)
    W2 = np.asarray(W2, np.float32)
    W3 = np.asarray(W3, np.float32)
    W1d = np.zeros((128, 64), np.float32)
    W2d = np.zeros((64, 32), np.float32)
    W3d = np.zeros((32, 4), np.float32)
    for l in range(4):
        W1d[l * 32 : (l + 1) * 32, l * 16 : (l + 1) * 16] = W1
        W2d[l * 16 : (l + 1) * 16, l * 8 : (l + 1) * 8] = W2
        W3d[l * 8 : (l + 1) * 8, l : l + 1] = W3
    b1d = np.tile(np.asarray(b1, np.float32), 4).reshape(64, 1)
    b2d = np.tile(np.asarray(b2, np.float32), 4).reshape(32, 1)
    b3d = np.tile(np.asarray(b3, np.float32), 4).reshape(4, 1)
    return W1d, W2d, W3d, b1d, b2d, b3d


_NC = None


def _get_nc():
    global _NC
    if _NC is None:
        _NC = build_nc()
    return _NC


def kernel(x, pos, batch, W1, b1, W2, b2, W3, b3):
    from concourse.bass_utils import run_bass_kernel_spmd

    x = np.ascontiguousarray(np.asarray(x, np.float32))
    pos = np.ascontiguousarray(np.asarray(pos, np.float32))
    W1d, W2d, W3d, b1d, b2d, b3d = _host_prep(W1, b1, W2, b2, W3, b3)
    eye128 = np.eye(128, dtype=np.float32)

    rows = S_CORE * NPTS
    in_maps = []
    for c in range(N_CORES):
        in_maps.append(
            {
                "pos": pos[c * rows : (c + 1) * rows],
                "x": x[c * rows : (c + 1) * rows],
                "W1d": W1d, "W2d": W2d, "W3d": W3d,
                "b1d": b1d, "b2d": b2d, "b3d": b3d,
                "eye128": eye128,
            }
        )

    nc = _get_nc()
    res = run_bass_kernel_spmd(nc, in_maps, list(range(N_CORES))).results
    weights = np.concatenate([res[c]["weights_out"] for c in range(N_CORES)], axis=0)
    indices = np.concatenate(
        [res[c]["indices_out"].astype(np.int32) for c in range(N_CORES)], axis=0
    )
    return weights, indices

